# revision 20
# baseline (speedup 1.0000x reference)
"""Multi-head attention (16 heads, RoPE, causal) for Trainium2, 8 NeuronCores.

Sharding: data-parallel over batch (2) x tensor-parallel over head groups (4),
one (batch, head-group-of-4) pair per core. Each core computes its 4 heads'
attention feature-major (transposed) and a partial output projection
outT = Wo_slice^T @ Y^T [1024, 2048]; the host sums the 4 partials per batch
and transposes back.

Device-side layout highlights:
  - Everything feature-major: Q^T/K^T [256, 2048] so the S^T (j-on-partition)
    matmuls need no on-chip transposes anywhere.
  - S^T tiles [j=128, i=512]: softmax denominator comes from a ones-column
    appended to V (O_aug row 64 = sum_j P[j,i]), so no partition reductions.
  - RoPE applied feature-major with a host-side even/odd split permutation of
    Wq/Wk columns (rows h*64+u: u<32 even dims, u>=32 odd dims), making the
    rotate-pairs step two 32-partition windowed multiplies per head.
  - exp() has no max-subtraction: logits are tiny for this problem family; a
    host-side spectral bound checks this and falls back to numpy otherwise.
  - Matmul operands in bf16 (fast weight load path), fp32 PSUM accumulation.
"""

import sys

sys.path.insert(0, "/opt/trn_rl_repo")
sys.path.insert(0, "/root/.axon_site")

import numpy as np

B, L, D = 2, 2048, 1024
H = 16                  # total heads
HD = 64                 # head dim
HPC = 4                 # heads per core
NCORES = 8
NT = 2                  # 128-row tiles per core of Q^T/K^T/Y^T (HPC*HD/128)
LC = L // 512           # 512-wide l chunks
KC = D // 128           # 128-deep contraction chunks over model dim
LT = L // 128           # 128-row l tiles

_cache = {}


def _build_nc(causal: bool):
    import contextlib

    import concourse.bass as bass
    import concourse.tile as tile
    from concourse import bacc, mybir

    F32 = mybir.dt.float32
    BF16 = mybir.dt.bfloat16
    EXP = mybir.ActivationFunctionType.Exp

    nc = bacc.Bacc("TRN2", target_bir_lowering=False, debug=False, num_devices=NCORES)

    xT = nc.dram_tensor("xT", [D, L], BF16, kind="ExternalInput")
    wq = nc.dram_tensor("wq", [D, 256], BF16, kind="ExternalInput")
    wk = nc.dram_tensor("wk", [D, 256], BF16, kind="ExternalInput")
    wv = nc.dram_tensor("wv", [D, 256], BF16, kind="ExternalInput")
    wo = nc.dram_tensor("wo", [256, D], BF16, kind="ExternalInput")
    cos128 = nc.dram_tensor("cos128", [128, L], BF16, kind="ExternalInput")
    srot128 = nc.dram_tensor("srot128", [128, L], BF16, kind="ExternalInput")
    mk4 = nc.dram_tensor("mk4", [128, 128], BF16, kind="ExternalInput")
    outT = nc.dram_tensor("outT", [D, L], F32, kind="ExternalOutput")

    with tile.TileContext(nc) as tc, \
         nc.allow_low_precision(reason="bf16 matmul pipeline by design"), \
         contextlib.ExitStack() as ctx:
        p_w = ctx.enter_context(tc.tile_pool(name="p_w", bufs=24))
        p_wo = ctx.enter_context(tc.tile_pool(name="p_wo", bufs=2))
        p_const = ctx.enter_context(tc.tile_pool(name="p_const", bufs=4))
        p_xt = ctx.enter_context(tc.tile_pool(name="p_xt", bufs=8))
        p_qt = ctx.enter_context(tc.tile_pool(name="p_qt", bufs=2))
        p_kt = ctx.enter_context(tc.tile_pool(name="p_kt", bufs=2))
        p_yt = ctx.enter_context(tc.tile_pool(name="p_yt", bufs=2))
        p_v = ctx.enter_context(tc.tile_pool(name="p_v", bufs=16))
        p_pt = ctx.enter_context(tc.tile_pool(name="p_pt", bufs=10))
        p_tmp = ctx.enter_context(tc.tile_pool(name="p_tmp", bufs=3))
        p_z = ctx.enter_context(tc.tile_pool(name="p_z", bufs=3))
        p_oc = ctx.enter_context(tc.tile_pool(name="p_oc", bufs=6))
        pp = ctx.enter_context(tc.tile_pool(name="pp", bufs=1, space="PSUM"))
        pst = ctx.enter_context(tc.tile_pool(name="pst", bufs=2, space="PSUM"))
        pso = ctx.enter_context(tc.tile_pool(name="pso", bufs=3, space="PSUM"))

        # ---- constant / weight / input loads ------------------------
        # interleave weight and x loads per contraction chunk so the first
        # projection matmuls can start as soon as chunk 0 lands
        wq_sb, wk_sb, wv_sb, x_sb = [], [], [], []
        for kc in range(KC):
            for srct, dst in ((wq, wq_sb), (wk, wk_sb)):
                w_t = p_w.tile([128, 256], BF16, tag="w")
                nc.sync.dma_start(out=w_t, in_=srct.ap()[kc * 128:(kc + 1) * 128, :])
                dst.append(w_t)
            x_t = p_xt.tile([128, L], BF16, tag="xt", name=f"xt{kc}")
            nc.sync.dma_start(out=x_t, in_=xT.ap()[kc * 128:(kc + 1) * 128, :])
            x_sb.append(x_t)
        for kc in range(KC):
            w_t = p_w.tile([128, 256], BF16, tag="w")
            nc.sync.dma_start(out=w_t, in_=wv.ap()[kc * 128:(kc + 1) * 128, :])
            wv_sb.append(w_t)
        cos_t = p_const.tile([128, L], BF16, tag="const")
        nc.sync.dma_start(out=cos_t, in_=cos128.ap())
        srot_t = p_const.tile([128, L], BF16, tag="const")
        nc.sync.dma_start(out=srot_t, in_=srot128.ap())
        mk_t = p_const.tile([128, 128], BF16, tag="tri")
        nc.sync.dma_start(out=mk_t, in_=mk4.ap())
        wo_sb = []
        for kc2 in range(2):
            wo_t = p_wo.tile([128, D], BF16, tag="wo")
            nc.sync.dma_start(out=wo_t, in_=wo.ap()[kc2 * 128:(kc2 + 1) * 128, :])
            wo_sb.append(wo_t)

        qt_sb = [p_qt.tile([128, L], BF16, tag="qt", name=f"qt{i}") for i in range(NT)]
        kt_sb = [p_kt.tile([128, L], BF16, tag="kt", name=f"kt{i}") for i in range(NT)]
        yt_sb = [p_yt.tile([128, L], BF16, tag="yt", name=f"yt{i}") for i in range(NT)]
        v_sb = [p_v.tile([128, HPC, 65], BF16, tag="vaug", name=f"vaug{i}")
                for i in range(LT)]

        # ---- Q^T / K^T projections + RoPE ---------------------------
        # evacuate psum via the (otherwise idle) scalar engine into bf16,
        # then do the rotate-pairs arithmetic as all-SBUF bf16 DVE ops
        # (2x mode). srot rows r hold +sin[r%32] (r%64<32) / -sin[r%32].
        def rope_evac(ps, trg, lc):
            sl = slice(lc * 512, (lc + 1) * 512)
            qraw = p_tmp.tile([128, 512], BF16, tag="qraw")
            nc.scalar.copy(qraw[:, :], ps[:, :])
            tmp = p_tmp.tile([128, 512], BF16, tag="tmp")
            for hh in range(2):
                b0 = hh * 64
                nc.vector.tensor_mul(tmp[b0:b0 + 32, :], qraw[b0 + 32:b0 + 64, :],
                                     srot_t[b0 + 32:b0 + 64, sl])
                nc.vector.tensor_mul(tmp[b0 + 32:b0 + 64, :], qraw[b0:b0 + 32, :],
                                     srot_t[b0:b0 + 32, sl])
            nc.vector.tensor_mul(trg[:, sl], qraw[:, :], cos_t[:, sl])
            nc.vector.tensor_add(trg[:, sl], trg[:, sl], tmp[:, :])

        # each weight tile is loaded into the PE array once and reused for
        # 2 back-to-back matmuls (one per 512-wide l chunk of the pair)
        for w_list, trg_list in ((wq_sb, qt_sb), (wk_sb, kt_sb)):
            for nt in range(NT):
                for lc0 in range(0, LC, 2):
                    ps_ab = [pp.tile([128, 512], F32, tag="pp", name=f"pj{i}")
                             for i in range(2)]
                    for kc in range(KC):
                        w_ap = w_list[kc][:, nt * 128:(nt + 1) * 128]
                        for i in range(2):
                            nc.tensor.matmul(
                                ps_ab[i][:, :], w_ap,
                                x_sb[kc][:, (lc0 + i) * 512:(lc0 + i + 1) * 512],
                                start=(kc == 0), stop=(kc == KC - 1))
                    for i in range(2):
                        rope_evac(ps_ab[i][:, :], trg_list[nt], lc0 + i)

        # ---- V (row-major) + ones column ----------------------------
        for lt in range(LT):
            ps = pp.tile([128, 256], F32, tag="pp")
            for kc in range(KC):
                nc.tensor.matmul(
                    ps[:, :], x_sb[kc][:, lt * 128:(lt + 1) * 128],
                    wv_sb[kc][:, :], start=(kc == 0), stop=(kc == KC - 1))
            va = v_sb[lt]
            nc.vector.memset(va[:, :, 64:65], 1.0)
            nc.vector.tensor_copy(
                va[:, :, 0:64], ps[:, :].rearrange("p (h v) -> p h v", h=HPC))

        # ---- attention (c outer so exp overlaps later projections;
        #       Wo columns for chunk c emitted right after c completes) ----
        for c in range(LC):
            for h in range(HPC):
                nt, r0 = h // 2, (h % 2) * 64
                csl = slice(c * 512, (c + 1) * 512)
                jmax = 4 * c + 3 if causal else LT - 1
                oaug = pso.tile([65, 512], F32, tag="oaug")

                # diagonal-strip tile j = 4c+k only has valid i-columns
                # >= 128k within this chunk; trim matmuls to that range
                def trim(j, c=c):
                    k = j - 4 * c
                    return 128 * k if (causal and k >= 0) else 0

                def emit_o(jp, pt, jmax=jmax, oaug=oaug, h=h):
                    for s in range(2):
                        j = 2 * jp + s
                        t = trim(j)
                        nc.tensor.matmul(
                            oaug[:, t:512], v_sb[j][:, h, :],
                            pt[:, s * 512 + t:(s + 1) * 512],
                            start=(j == 0), stop=(j == jmax))

                lagq = []
                for jp in range((jmax + 1) // 2):
                    st = pst.tile([128, 1024], F32, tag="st")
                    for s in range(2):
                        j = 2 * jp + s
                        t = trim(j)
                        nc.tensor.matmul(
                            st[:, s * 512 + t:(s + 1) * 512],
                            kt_sb[nt][r0:r0 + 64, j * 128:(j + 1) * 128],
                            qt_sb[nt][r0:r0 + 64, c * 512 + t:(c + 1) * 512],
                            start=True, stop=True)
                    pt = p_pt.tile([128, 1024], BF16, tag="pt")
                    t0 = trim(2 * jp)
                    nc.scalar.activation(pt[:, t0:], st[:, t0:], EXP)
                    if causal:
                        for s in range(2):
                            k = 2 * jp + s - 4 * c
                            if k >= 0:
                                sl = slice(s * 512 + 128 * k, s * 512 + 128 * (k + 1))
                                nc.vector.tensor_mul(pt[:, sl], pt[:, sl], mk_t[:, :])
                    lagq.append((jp, pt))
                    if len(lagq) > 2:
                        emit_o(*lagq.pop(0))
                for args in lagq:
                    emit_o(*args)
                zs = p_z.tile([1, 512], F32, tag="zs")
                nc.vector.tensor_copy(zs[0:1, :], oaug[64:65, :])
                zrow = p_z.tile([1, 512], F32, tag="zrow")
                nc.vector.reciprocal_approx_fast(zrow[0:1, :], zs[0:1, :])
                zb = p_z.tile([64, 512], F32, tag="zb")
                nc.gpsimd.partition_broadcast(zb[:, :], zrow[0:1, :])
                nc.vector.tensor_mul(yt_sb[nt][r0:r0 + 64, csl],
                                     oaug[0:64, :], zb[:, :])

            # ---- output projection for this chunk's columns ---------
            for ot in range(8):
                ps = pp.tile([128, 512], F32, tag="pp")
                for kc2 in range(2):
                    nc.tensor.matmul(
                        ps[:, :], wo_sb[kc2][:, ot * 128:(ot + 1) * 128],
                        yt_sb[kc2][:, c * 512:(c + 1) * 512],
                        start=(kc2 == 0), stop=(kc2 == 1))
                oc = p_oc.tile([128, 512], F32, tag="oc")
                if c == LC - 1 and ot % 2 == 1:
                    nc.scalar.copy(oc[:, :], ps[:, :])
                else:
                    nc.vector.tensor_copy(oc[:, :], ps[:, :])
                nc.sync.dma_start(
                    out=outT.ap()[ot * 128:(ot + 1) * 128, c * 512:(c + 1) * 512],
                    in_=oc[:, :])

    nc.compile()
    return nc


def _get_nc(causal: bool):
    key = "causal" if causal else "dense"
    if key not in _cache:
        _cache[key] = _build_nc(causal)
    return _cache[key]


def _rope_np(x):
    d, s = x.shape[-1], x.shape[-2]
    ts = np.arange(0, d, 2, dtype=np.float32)
    inv = 10000.0 ** (-ts / d)
    grid = np.arange(s, dtype=np.float32)[:, None] * inv[None, :]
    sin = np.repeat(np.sin(grid), 2, axis=-1)
    cos = np.repeat(np.cos(grid), 2, axis=-1)
    x1, x2 = x[..., ::2], x[..., 1::2]
    xs = np.stack([-x2, x1], axis=-1).reshape(x.shape)
    return x * cos + xs * sin


def _reference_np(x, mask, Wq, Wk, Wv, Wo):
    b, l, d = x.shape
    h, k_sz = H, D // H
    split = lambda t: t.reshape(b, l, h, k_sz).transpose(0, 2, 1, 3)
    q = split((x @ Wq) / np.sqrt(np.float32(d)))
    q = _rope_np(q)
    k = _rope_np(split(x @ Wk))
    v = split(x @ Wv)
    logits = np.einsum("bhik,bhjk->bhij", q, k) + mask
    m = logits.max(axis=-1, keepdims=True)
    p = np.exp(logits - m)
    a = p / p.sum(axis=-1, keepdims=True)
    y = np.einsum("bhij,bhjv->bhiv", a, v)
    y = y.transpose(0, 2, 1, 3).reshape(b, l, d)
    return (y @ Wo).astype(np.float32)


def _spectral_norm(w, iters=12):
    rng = np.random.default_rng(0)
    v = rng.standard_normal(w.shape[1]).astype(np.float32)
    for _ in range(iters):
        u = w @ v
        u /= (np.linalg.norm(u) + 1e-30)
        v = w.T @ u
        nv = np.linalg.norm(v)
        v /= (nv + 1e-30)
    return float(nv)


def _host_consts():
    inv = 10000.0 ** (-np.arange(0, HD, 2, dtype=np.float32) / HD)
    grid = np.arange(L, dtype=np.float32)[None, :] * inv[:, None]   # [32, L]
    cos32 = np.cos(grid).astype(np.float32)
    sin32 = np.sin(grid).astype(np.float32)
    cos128 = np.ascontiguousarray(np.tile(cos32, (4, 1)))
    # srot rows r: +sin[r%32] for r%64 < 32, -sin[r%32] otherwise
    srot128 = np.ascontiguousarray(
        np.tile(np.concatenate([sin32, -sin32], axis=0), (2, 1)))
    tri = (np.arange(128)[None, :] >= np.arange(128)[:, None]).astype(np.float32)
    return cos128, srot128, np.ascontiguousarray(tri)


def _make_in_maps(x, Wq, Wk, Wv, Wo):
    import ml_dtypes
    bf16 = ml_dtypes.bfloat16

    cos128, srot128, mk4 = _host_consts()
    cos128 = cos128.astype(bf16)
    srot128 = srot128.astype(bf16)
    mk4 = mk4.astype(bf16)
    perm = np.concatenate([np.arange(0, 64, 2), np.arange(1, 64, 2)])
    Wq_s = (Wq / np.sqrt(np.float32(D))).astype(np.float32)
    in_maps = []
    for core in range(NCORES):
        bi, g = core // 4, core % 4
        xT_b = np.ascontiguousarray(x[bi].T.astype(bf16))
        wq_c = np.empty((D, 256), np.float32)
        wk_c = np.empty((D, 256), np.float32)
        for hh in range(HPC):
            h_abs = g * HPC + hh
            wq_c[:, hh * 64:(hh + 1) * 64] = Wq_s[:, h_abs * 64:(h_abs + 1) * 64][:, perm]
            wk_c[:, hh * 64:(hh + 1) * 64] = Wk[:, h_abs * 64:(h_abs + 1) * 64][:, perm]
        in_maps.append({
            "xT": xT_b,
            "wq": wq_c.astype(bf16),
            "wk": wk_c.astype(bf16),
            "wv": np.ascontiguousarray(Wv[:, g * 256:(g + 1) * 256].astype(bf16)),
            "wo": np.ascontiguousarray(Wo[g * 256:(g + 1) * 256, :].astype(bf16)),
            "cos128": cos128, "srot128": srot128, "mk4": mk4,
        })
    return in_maps


def kernel(x, mask, Wq, Wk, Wv, Wo):
    from concourse.bass_utils import run_bass_kernel_spmd

    x = np.asarray(x, dtype=np.float32)
    mask = np.asarray(mask, dtype=np.float32)
    Wq = np.asarray(Wq, dtype=np.float32)
    Wk = np.asarray(Wk, dtype=np.float32)
    Wv = np.asarray(Wv, dtype=np.float32)
    Wo = np.asarray(Wo, dtype=np.float32)

    # classify the mask
    m = mask.reshape(L, L)
    tril = np.tril(np.ones((L, L), dtype=bool))
    visible = m > -1e6
    if np.array_equal(visible, tril) and not m[tril].any():
        causal = True
    elif not m.any():
        causal = False
    else:
        return _reference_np(x, mask, Wq, Wk, Wv, Wo)

    # overflow guard for the no-max-subtraction softmax
    xr = float(np.sqrt((x * x).sum(axis=2).max()))
    bound = (xr * _spectral_norm(Wq) / np.sqrt(D)) * (xr * _spectral_norm(Wk))
    if bound > 60.0:
        return _reference_np(x, mask, Wq, Wk, Wv, Wo)

    in_maps = _make_in_maps(x, Wq, Wk, Wv, Wo)
    nc = _get_nc(causal)
    res = run_bass_kernel_spmd(nc, in_maps, core_ids=list(range(NCORES)))

    out = np.empty((B, L, D), dtype=np.float32)
    for bi in range(B):
        acc = res.results[bi * 4]["outT"].copy()
        for g in range(1, 4):
            acc += res.results[bi * 4 + g]["outT"]
        out[bi] = acc.T
    return out


# revision 21
# speedup vs baseline: 1.0034x; 1.0034x over previous
"""Multi-head attention (16 heads, RoPE, causal) for Trainium2, 8 NeuronCores.

Sharding: data-parallel over batch (2) x tensor-parallel over head groups (4),
one (batch, head-group-of-4) pair per core. Each core computes its 4 heads'
attention feature-major (transposed) and a partial output projection
outT = Wo_slice^T @ Y^T [1024, 2048]; the host sums the 4 partials per batch
and transposes back.

Device-side layout highlights:
  - Everything feature-major: Q^T/K^T [256, 2048] so the S^T (j-on-partition)
    matmuls need no on-chip transposes anywhere.
  - S^T tiles [j=128, i=512]: softmax denominator comes from a ones-column
    appended to V (O_aug row 64 = sum_j P[j,i]), so no partition reductions.
  - RoPE applied feature-major with a host-side even/odd split permutation of
    Wq/Wk columns (rows h*64+u: u<32 even dims, u>=32 odd dims), making the
    rotate-pairs step two 32-partition windowed multiplies per head.
  - exp() has no max-subtraction: logits are tiny for this problem family; a
    host-side spectral bound checks this and falls back to numpy otherwise.
  - Matmul operands in bf16 (fast weight load path), fp32 PSUM accumulation.
"""

import sys

sys.path.insert(0, "/opt/trn_rl_repo")
sys.path.insert(0, "/root/.axon_site")

import numpy as np

B, L, D = 2, 2048, 1024
H = 16                  # total heads
HD = 64                 # head dim
HPC = 4                 # heads per core
NCORES = 8
NT = 2                  # 128-row tiles per core of Q^T/K^T/Y^T (HPC*HD/128)
LC = L // 512           # 512-wide l chunks
KC = D // 128           # 128-deep contraction chunks over model dim
LT = L // 128           # 128-row l tiles

_cache = {}


def _build_nc(causal: bool):
    import contextlib

    import concourse.bass as bass
    import concourse.tile as tile
    from concourse import bacc, mybir

    F32 = mybir.dt.float32
    BF16 = mybir.dt.bfloat16
    EXP = mybir.ActivationFunctionType.Exp

    nc = bacc.Bacc("TRN2", target_bir_lowering=False, debug=False, num_devices=NCORES)

    xT = nc.dram_tensor("xT", [D, L], BF16, kind="ExternalInput")
    wq = nc.dram_tensor("wq", [D, 256], BF16, kind="ExternalInput")
    wk = nc.dram_tensor("wk", [D, 256], BF16, kind="ExternalInput")
    wv = nc.dram_tensor("wv", [D, 256], BF16, kind="ExternalInput")
    wo = nc.dram_tensor("wo", [256, D], BF16, kind="ExternalInput")
    cos128 = nc.dram_tensor("cos128", [128, L], BF16, kind="ExternalInput")
    srot128 = nc.dram_tensor("srot128", [128, L], BF16, kind="ExternalInput")
    mk4 = nc.dram_tensor("mk4", [128, 128], BF16, kind="ExternalInput")
    outT = nc.dram_tensor("outT", [D, L], F32, kind="ExternalOutput")

    with tile.TileContext(nc) as tc, \
         nc.allow_low_precision(reason="bf16 matmul pipeline by design"), \
         contextlib.ExitStack() as ctx:
        p_w = ctx.enter_context(tc.tile_pool(name="p_w", bufs=24))
        p_wo = ctx.enter_context(tc.tile_pool(name="p_wo", bufs=2))
        p_const = ctx.enter_context(tc.tile_pool(name="p_const", bufs=4))
        p_xt = ctx.enter_context(tc.tile_pool(name="p_xt", bufs=8))
        p_qt = ctx.enter_context(tc.tile_pool(name="p_qt", bufs=2))
        p_kt = ctx.enter_context(tc.tile_pool(name="p_kt", bufs=2))
        p_yt = ctx.enter_context(tc.tile_pool(name="p_yt", bufs=2))
        p_v = ctx.enter_context(tc.tile_pool(name="p_v", bufs=16))
        p_pt = ctx.enter_context(tc.tile_pool(name="p_pt", bufs=10))
        p_tmp = ctx.enter_context(tc.tile_pool(name="p_tmp", bufs=3))
        p_z = ctx.enter_context(tc.tile_pool(name="p_z", bufs=3))
        p_oc = ctx.enter_context(tc.tile_pool(name="p_oc", bufs=6))
        pp = ctx.enter_context(tc.tile_pool(name="pp", bufs=2, space="PSUM"))
        pst = ctx.enter_context(tc.tile_pool(name="pst", bufs=2, space="PSUM"))
        pso = ctx.enter_context(tc.tile_pool(name="pso", bufs=2, space="PSUM"))

        # ---- constant / weight / input loads ------------------------
        # interleave weight and x loads per contraction chunk so the first
        # projection matmuls can start as soon as chunk 0 lands
        wq_sb, wk_sb, wv_sb, x_sb = [], [], [], []
        for kc in range(KC):
            for srct, dst in ((wq, wq_sb), (wk, wk_sb)):
                w_t = p_w.tile([128, 256], BF16, tag="w")
                nc.sync.dma_start(out=w_t, in_=srct.ap()[kc * 128:(kc + 1) * 128, :])
                dst.append(w_t)
            x_t = p_xt.tile([128, L], BF16, tag="xt", name=f"xt{kc}")
            nc.sync.dma_start(out=x_t, in_=xT.ap()[kc * 128:(kc + 1) * 128, :])
            x_sb.append(x_t)
        for kc in range(KC):
            w_t = p_w.tile([128, 256], BF16, tag="w")
            nc.sync.dma_start(out=w_t, in_=wv.ap()[kc * 128:(kc + 1) * 128, :])
            wv_sb.append(w_t)
        cos_t = p_const.tile([128, L], BF16, tag="const")
        nc.sync.dma_start(out=cos_t, in_=cos128.ap())
        srot_t = p_const.tile([128, L], BF16, tag="const")
        nc.sync.dma_start(out=srot_t, in_=srot128.ap())
        mk_t = p_const.tile([128, 128], BF16, tag="tri")
        nc.sync.dma_start(out=mk_t, in_=mk4.ap())
        wo_sb = []
        for kc2 in range(2):
            wo_t = p_wo.tile([128, D], BF16, tag="wo")
            nc.sync.dma_start(out=wo_t, in_=wo.ap()[kc2 * 128:(kc2 + 1) * 128, :])
            wo_sb.append(wo_t)

        qt_sb = [p_qt.tile([128, L], BF16, tag="qt", name=f"qt{i}") for i in range(NT)]
        kt_sb = [p_kt.tile([128, L], BF16, tag="kt", name=f"kt{i}") for i in range(NT)]
        yt_sb = [p_yt.tile([128, L], BF16, tag="yt", name=f"yt{i}") for i in range(NT)]
        v_sb = [p_v.tile([128, HPC, 65], BF16, tag="vaug", name=f"vaug{i}")
                for i in range(LT)]

        # ---- Q^T / K^T projections + RoPE ---------------------------
        # evacuate psum via the (otherwise idle) scalar engine into bf16,
        # then do the rotate-pairs arithmetic as all-SBUF bf16 DVE ops
        # (2x mode). srot rows r hold +sin[r%32] (r%64<32) / -sin[r%32].
        def rope_evac(ps, trg, lc):
            sl = slice(lc * 512, (lc + 1) * 512)
            qraw = p_tmp.tile([128, 512], BF16, tag="qraw")
            nc.scalar.copy(qraw[:, :], ps[:, :])
            tmp = p_tmp.tile([128, 512], BF16, tag="tmp")
            for hh in range(2):
                b0 = hh * 64
                nc.vector.tensor_mul(tmp[b0:b0 + 32, :], qraw[b0 + 32:b0 + 64, :],
                                     srot_t[b0 + 32:b0 + 64, sl])
                nc.vector.tensor_mul(tmp[b0 + 32:b0 + 64, :], qraw[b0:b0 + 32, :],
                                     srot_t[b0:b0 + 32, sl])
            nc.vector.tensor_mul(trg[:, sl], qraw[:, :], cos_t[:, sl])
            nc.vector.tensor_add(trg[:, sl], trg[:, sl], tmp[:, :])

        # each weight tile is loaded into the PE array once and reused for
        # 2 back-to-back matmuls (one per 512-wide l chunk of the pair)
        for w_list, trg_list in ((wq_sb, qt_sb), (wk_sb, kt_sb)):
            for nt in range(NT):
                for lc0 in range(0, LC, 2):
                    ps_ab = [pp.tile([128, 512], F32, tag="pp", name=f"pj{i}")
                             for i in range(2)]
                    for kc in range(KC):
                        w_ap = w_list[kc][:, nt * 128:(nt + 1) * 128]
                        for i in range(2):
                            nc.tensor.matmul(
                                ps_ab[i][:, :], w_ap,
                                x_sb[kc][:, (lc0 + i) * 512:(lc0 + i + 1) * 512],
                                start=(kc == 0), stop=(kc == KC - 1))
                    for i in range(2):
                        rope_evac(ps_ab[i][:, :], trg_list[nt], lc0 + i)

        # ---- V (row-major) + ones column ----------------------------
        for lt in range(LT):
            ps = pp.tile([128, 256], F32, tag="pp")
            for kc in range(KC):
                nc.tensor.matmul(
                    ps[:, :], x_sb[kc][:, lt * 128:(lt + 1) * 128],
                    wv_sb[kc][:, :], start=(kc == 0), stop=(kc == KC - 1))
            va = v_sb[lt]
            nc.vector.memset(va[:, :, 64:65], 1.0)
            nc.vector.tensor_copy(
                va[:, :, 0:64], ps[:, :].rearrange("p (h v) -> p h v", h=HPC))

        # ---- attention (c outer so exp overlaps later projections;
        #       Wo columns for chunk c emitted right after c completes) ----
        for c in range(LC):
            for h in range(HPC):
                nt, r0 = h // 2, (h % 2) * 64
                csl = slice(c * 512, (c + 1) * 512)
                jmax = 4 * c + 3 if causal else LT - 1
                oaug = pso.tile([65, 512], F32, tag="oaug")

                # diagonal-strip tile j = 4c+k only has valid i-columns
                # >= 128k within this chunk; trim matmuls to that range
                def trim(j, c=c):
                    k = j - 4 * c
                    return 128 * k if (causal and k >= 0) else 0

                def emit_o(jp, pt, jmax=jmax, oaug=oaug, h=h):
                    for s in range(2):
                        j = 2 * jp + s
                        t = trim(j)
                        nc.tensor.matmul(
                            oaug[:, t:512], v_sb[j][:, h, :],
                            pt[:, s * 512 + t:(s + 1) * 512],
                            start=(j == 0), stop=(j == jmax))

                lagq = []
                for jp in range((jmax + 1) // 2):
                    st = pst.tile([128, 1024], F32, tag="st")
                    for s in range(2):
                        j = 2 * jp + s
                        t = trim(j)
                        nc.tensor.matmul(
                            st[:, s * 512 + t:(s + 1) * 512],
                            kt_sb[nt][r0:r0 + 64, j * 128:(j + 1) * 128],
                            qt_sb[nt][r0:r0 + 64, c * 512 + t:(c + 1) * 512],
                            start=True, stop=True)
                    pt = p_pt.tile([128, 1024], BF16, tag="pt")
                    t0 = trim(2 * jp)
                    nc.scalar.activation(pt[:, t0:], st[:, t0:], EXP)
                    if causal:
                        for s in range(2):
                            k = 2 * jp + s - 4 * c
                            if k >= 0:
                                sl = slice(s * 512 + 128 * k, s * 512 + 128 * (k + 1))
                                nc.vector.tensor_mul(pt[:, sl], pt[:, sl], mk_t[:, :])
                    lagq.append((jp, pt))
                    if len(lagq) > 2:
                        emit_o(*lagq.pop(0))
                for args in lagq:
                    emit_o(*args)
                zs = p_z.tile([1, 512], F32, tag="zs")
                nc.vector.tensor_copy(zs[0:1, :], oaug[64:65, :])
                zrow = p_z.tile([1, 512], F32, tag="zrow")
                nc.vector.reciprocal_approx_fast(zrow[0:1, :], zs[0:1, :])
                zb = p_z.tile([64, 512], F32, tag="zb")
                nc.gpsimd.partition_broadcast(zb[:, :], zrow[0:1, :])
                nc.vector.tensor_mul(yt_sb[nt][r0:r0 + 64, csl],
                                     oaug[0:64, :], zb[:, :])

            # ---- output projection for this chunk's columns ---------
            for ot in range(8):
                ps = pp.tile([128, 512], F32, tag="pp")
                for kc2 in range(2):
                    nc.tensor.matmul(
                        ps[:, :], wo_sb[kc2][:, ot * 128:(ot + 1) * 128],
                        yt_sb[kc2][:, c * 512:(c + 1) * 512],
                        start=(kc2 == 0), stop=(kc2 == 1))
                oc = p_oc.tile([128, 512], F32, tag="oc")
                if c == LC - 1 and ot % 2 == 1:
                    nc.scalar.copy(oc[:, :], ps[:, :])
                else:
                    nc.vector.tensor_copy(oc[:, :], ps[:, :])
                nc.sync.dma_start(
                    out=outT.ap()[ot * 128:(ot + 1) * 128, c * 512:(c + 1) * 512],
                    in_=oc[:, :])

    nc.compile()
    return nc


def _get_nc(causal: bool):
    key = "causal" if causal else "dense"
    if key not in _cache:
        _cache[key] = _build_nc(causal)
    return _cache[key]


def _rope_np(x):
    d, s = x.shape[-1], x.shape[-2]
    ts = np.arange(0, d, 2, dtype=np.float32)
    inv = 10000.0 ** (-ts / d)
    grid = np.arange(s, dtype=np.float32)[:, None] * inv[None, :]
    sin = np.repeat(np.sin(grid), 2, axis=-1)
    cos = np.repeat(np.cos(grid), 2, axis=-1)
    x1, x2 = x[..., ::2], x[..., 1::2]
    xs = np.stack([-x2, x1], axis=-1).reshape(x.shape)
    return x * cos + xs * sin


def _reference_np(x, mask, Wq, Wk, Wv, Wo):
    b, l, d = x.shape
    h, k_sz = H, D // H
    split = lambda t: t.reshape(b, l, h, k_sz).transpose(0, 2, 1, 3)
    q = split((x @ Wq) / np.sqrt(np.float32(d)))
    q = _rope_np(q)
    k = _rope_np(split(x @ Wk))
    v = split(x @ Wv)
    logits = np.einsum("bhik,bhjk->bhij", q, k) + mask
    m = logits.max(axis=-1, keepdims=True)
    p = np.exp(logits - m)
    a = p / p.sum(axis=-1, keepdims=True)
    y = np.einsum("bhij,bhjv->bhiv", a, v)
    y = y.transpose(0, 2, 1, 3).reshape(b, l, d)
    return (y @ Wo).astype(np.float32)


def _spectral_norm(w, iters=12):
    rng = np.random.default_rng(0)
    v = rng.standard_normal(w.shape[1]).astype(np.float32)
    for _ in range(iters):
        u = w @ v
        u /= (np.linalg.norm(u) + 1e-30)
        v = w.T @ u
        nv = np.linalg.norm(v)
        v /= (nv + 1e-30)
    return float(nv)


def _host_consts():
    inv = 10000.0 ** (-np.arange(0, HD, 2, dtype=np.float32) / HD)
    grid = np.arange(L, dtype=np.float32)[None, :] * inv[:, None]   # [32, L]
    cos32 = np.cos(grid).astype(np.float32)
    sin32 = np.sin(grid).astype(np.float32)
    cos128 = np.ascontiguousarray(np.tile(cos32, (4, 1)))
    # srot rows r: +sin[r%32] for r%64 < 32, -sin[r%32] otherwise
    srot128 = np.ascontiguousarray(
        np.tile(np.concatenate([sin32, -sin32], axis=0), (2, 1)))
    tri = (np.arange(128)[None, :] >= np.arange(128)[:, None]).astype(np.float32)
    return cos128, srot128, np.ascontiguousarray(tri)


def _make_in_maps(x, Wq, Wk, Wv, Wo):
    import ml_dtypes
    bf16 = ml_dtypes.bfloat16

    cos128, srot128, mk4 = _host_consts()
    cos128 = cos128.astype(bf16)
    srot128 = srot128.astype(bf16)
    mk4 = mk4.astype(bf16)
    perm = np.concatenate([np.arange(0, 64, 2), np.arange(1, 64, 2)])
    Wq_s = (Wq / np.sqrt(np.float32(D))).astype(np.float32)
    in_maps = []
    for core in range(NCORES):
        bi, g = core // 4, core % 4
        xT_b = np.ascontiguousarray(x[bi].T.astype(bf16))
        wq_c = np.empty((D, 256), np.float32)
        wk_c = np.empty((D, 256), np.float32)
        for hh in range(HPC):
            h_abs = g * HPC + hh
            wq_c[:, hh * 64:(hh + 1) * 64] = Wq_s[:, h_abs * 64:(h_abs + 1) * 64][:, perm]
            wk_c[:, hh * 64:(hh + 1) * 64] = Wk[:, h_abs * 64:(h_abs + 1) * 64][:, perm]
        in_maps.append({
            "xT": xT_b,
            "wq": wq_c.astype(bf16),
            "wk": wk_c.astype(bf16),
            "wv": np.ascontiguousarray(Wv[:, g * 256:(g + 1) * 256].astype(bf16)),
            "wo": np.ascontiguousarray(Wo[g * 256:(g + 1) * 256, :].astype(bf16)),
            "cos128": cos128, "srot128": srot128, "mk4": mk4,
        })
    return in_maps


def kernel(x, mask, Wq, Wk, Wv, Wo):
    from concourse.bass_utils import run_bass_kernel_spmd

    x = np.asarray(x, dtype=np.float32)
    mask = np.asarray(mask, dtype=np.float32)
    Wq = np.asarray(Wq, dtype=np.float32)
    Wk = np.asarray(Wk, dtype=np.float32)
    Wv = np.asarray(Wv, dtype=np.float32)
    Wo = np.asarray(Wo, dtype=np.float32)

    # classify the mask
    m = mask.reshape(L, L)
    tril = np.tril(np.ones((L, L), dtype=bool))
    visible = m > -1e6
    if np.array_equal(visible, tril) and not m[tril].any():
        causal = True
    elif not m.any():
        causal = False
    else:
        return _reference_np(x, mask, Wq, Wk, Wv, Wo)

    # overflow guard for the no-max-subtraction softmax
    xr = float(np.sqrt((x * x).sum(axis=2).max()))
    bound = (xr * _spectral_norm(Wq) / np.sqrt(D)) * (xr * _spectral_norm(Wk))
    if bound > 60.0:
        return _reference_np(x, mask, Wq, Wk, Wv, Wo)

    in_maps = _make_in_maps(x, Wq, Wk, Wv, Wo)
    nc = _get_nc(causal)
    res = run_bass_kernel_spmd(nc, in_maps, core_ids=list(range(NCORES)))

    out = np.empty((B, L, D), dtype=np.float32)
    for bi in range(B):
        acc = res.results[bi * 4]["outT"].copy()
        for g in range(1, 4):
            acc += res.results[bi * 4 + g]["outT"]
        out[bi] = acc.T
    return out
